# revision 29
# baseline (speedup 1.0000x reference)
"""DigitCaps dynamic-routing kernel for Trainium2 (8 NeuronCores, Bass/Tile).

Problem: B=256, IN_CAPS=3200, IN_DIM=8, OUT_CAPS=8, OUT_DIM=16, 3 routing
iterations.  Data-parallel over batch: 32 batches per core.

Per core (v2 design):
  - u_hat is created ONCE per 16-batch half via full-K=128 matmuls over a
    host-prepared block-diagonal x operand ([jm=128-partition, (b, i)]
    layout, bf16), then XBAR-transposed into the resident i-layout copy
    u_res [i, t, b, jm].
  - it=2 a-pass uses the creation-staging jm-layout tiles directly;
    it=3 rebuilds jm-layout tiles from u_res via reverse XBAR transposes
    (this replaces the baseline's second full creation pass: saves the
    second xblk DMA read, 400 matmuls and 200 PSUM->SBUF copies).
  - squash runs entirely in [jm, b] layout: sq comes from a mask matmul
    (contract m within each j block), the j-indexed scale is re-broadcast
    across jm partitions with the transposed mask matmul.  No identity
    transposes, no per-squash XBAR.
  - PE emission is software-pipelined: creation(t) | a-pass(t-1) |
    s-pass(t-2) so PE never waits on the copy/softmax chains.
  - PSUM->SBUF creation copies rotate over Pool/Pool/DVE/Act; XBAR and
    xblk DMAs alternate between the SP and Act hardware DGE queues.
"""

import sys

if "/opt/trn_rl_repo" not in sys.path:
    sys.path.insert(0, "/opt/trn_rl_repo")

import ml_dtypes
import numpy as np

import bass_rust
import concourse.bass as bass
import concourse.mybir as mybir
import concourse.tile as tile
from concourse._compat import with_exitstack
from concourse.bass_utils import run_bass_kernel_spmd
from concourse.vector_clock import ScopedClock

# ---------------------------------------------------------------------------
# Walrus on this toolchain rejects multi-wait CTRL instructions;
# TileContext's tail drain aggregates one wait per outstanding semaphore.
# Split the waits across consecutive SP drains.
_TILE_PATCHED = False


def _drain_and_barrier_split(self, tick_clock, wait_clock):
    drain_inst = self.nc.sync.drain()
    wait_clock.add_sem_waits(
        drain_inst.ins, ScopedClock({None: tick_clock.global_clock})
    )
    mi = drain_inst.ins
    waits = list(mi.sync_info.on_wait) if mi.sync_info else []
    if len(waits) > 1:
        si = mi.sync_info
        si.on_wait = waits[:1]
        mi.sync_info = si
        for i in range(1, len(waits)):
            extra = self.nc.sync.drain().ins
            extra.sync_info = bass_rust.SyncInfo(
                on_wait=waits[i : i + 1], on_update=[]
            )
    self.nc.all_engine_barrier()
    assert self.sems is not None
    popped = self.nc._tile_sem_poison_stack.pop()
    assert popped is self._sem_poison
    self.nc.clear_and_free_semaphores(list(self.sems.allocated().values()))
    self.nc.all_engine_barrier()


def _patch_tile():
    global _TILE_PATCHED
    if not _TILE_PATCHED:
        tile.TileContext._drain_and_barrier = _drain_and_barrier_split
        _TILE_PATCHED = True


_SW_COUNT = [0]


def _split_waits(nc):
    """This walrus build allows one sync wait per instruction: hoist extra
    waits onto same-engine NoOp carriers placed just before."""
    for f in nc.m.functions:
        for blk in f.blocks:
            insts = blk.instructions
            if not any(
                inst.sync_info and len(inst.sync_info.on_wait) > 1
                for inst in insts
            ):
                continue
            new = []
            for inst in insts:
                si = inst.sync_info
                waits = list(si.on_wait) if si else []
                if len(waits) > 1:
                    for w in waits[:-1]:
                        _SW_COUNT[0] += 1
                        car = mybir.InstNoOp(
                            name=f"I-sw{_SW_COUNT[0]}", engine=inst.engine
                        )
                        car.sync_info = bass_rust.SyncInfo(
                            on_wait=[w], on_update=[]
                        )
                        new.append(car)
                    si.on_wait = waits[-1:]
                    inst.sync_info = si
                new.append(inst)
            insts[:] = new


# ---------------------------------------------------------------------------
B, I, N, J, M = 256, 3200, 8, 8, 16
JM = J * M  # 128
N_CORES = 8
B_C = B // N_CORES  # 32
T = I // 128  # 25 i-tiles

IP = 16  # i's packed per K-chunk (K = IP*N = 128, uniform row group)
KR = IP * N  # 128 K-rows per chunk
H = I // IP  # 200
CH_T = 128 // IP  # 8 creation chunks per 128-i tile

F32 = mybir.dt.float32
BF16 = mybir.dt.bfloat16


@with_exitstack
def build_kernel(ctx, tc, outs, ins, b_c=B_C, half=16, reps=1, stage=3,
                 it3_mode="xbar", xb_mode="chip", scr_bufs=2, a3_lag=2):
    """stage: 1=creation only, 2=+a+softmax, 3=full (timing ablation)."""
    nc = tc.nc
    (v_out,) = outs
    if xb_mode == "dram":
        (wcr_d, xt_d, mask_d, ipm_d, xblk_d) = ins
    else:
        (wcr_d, xt_d, mask_d, ipm_d) = ins
    n_half = b_c // half

    const = ctx.enter_context(tc.tile_pool(name="const", bufs=1))
    res = ctx.enter_context(tc.tile_pool(name="res", bufs=1))
    scr = ctx.enter_context(tc.tile_pool(name="scr", bufs=scr_bufs))
    scr3 = ctx.enter_context(tc.tile_pool(name="scr3", bufs=3))
    xs = ctx.enter_context(tc.tile_pool(name="xs", bufs=3))
    sm = ctx.enter_context(tc.tile_pool(name="sm", bufs=3))
    small = ctx.enter_context(tc.tile_pool(name="small", bufs=2))
    vbp = ctx.enter_context(tc.tile_pool(name="vbp", bufs=1))
    smx = ctx.enter_context(tc.tile_pool(name="smx", bufs=1))
    ps = ctx.enter_context(tc.tile_pool(name="ps", bufs=1, space="PSUM"))
    psS = ctx.enter_context(tc.tile_pool(name="psS", bufs=1, space="PSUM"))
    ps2 = ctx.enter_context(tc.tile_pool(name="ps2", bufs=2, space="PSUM"))
    ps3 = ctx.enter_context(tc.tile_pool(name="ps3", bufs=2, space="PSUM"))

    # Resident constants (K = 128 rows; all matmul bases stay 0 -- any mix
    # of stationary base partitions crashes this hardware).
    wcr = const.tile([128, H, JM], BF16)
    for k in range(4):
        eng = nc.sync if k % 2 == 0 else nc.scalar
        nc_q = H // 4
        eng.dma_start(
            wcr[:, k * nc_q : (k + 1) * nc_q, :],
            wcr_d[:, k * nc_q : (k + 1) * nc_q, :],
        )
    xt = const.tile([128, H, b_c], BF16)
    for k in range(2):
        eng = nc.sync if k % 2 == 0 else nc.scalar
        nc_q = H // 2
        eng.dma_start(
            xt[:, k * nc_q : (k + 1) * nc_q, :],
            xt_d[:, k * nc_q : (k + 1) * nc_q, :],
        )
    mask_rep = const.tile([JM, J], BF16)
    nc.sync.dma_start(mask_rep[:], mask_d[:])
    maskT = const.tile([J, JM], BF16)
    nc.sync.dma_start(maskT[:], mask_d[:].rearrange("a b -> b a"))
    if xb_mode == "chip2":
        ipm = const.tile([KR, half, IP], BF16)
        ipm_src = bass.AP(
            ipm_d.tensor, ipm_d.offset,
            [ipm_d.ap[0], [0, half], ipm_d.ap[1]],
        )
        nc.sync.dma_start(ipm[:], ipm_src)
    else:
        ipm = const.tile([KR, IP], BF16)
        nc.sync.dma_start(ipm[:], ipm_d[:])
    lg_res = const.tile([128, T, half, J], BF16)

    def squash_jm(s_src, nb, scale, s_src2=None):
        """s_src [JM, nb] f32 (PSUM/SBUF) (+ optional second PSUM operand)
        -> v_jm [JM, nb] f32 SBUF (squashed)."""
        s_sb = small.tile([JM, nb], F32, tag="s_sb")
        if s_src2 is not None:
            nc.vector.tensor_copy(s_sb[:], s_src)
            nc.vector.tensor_tensor(
                s_sb[:], s_sb[:], s_src2, mybir.AluOpType.add
            )
            if scale != 1.0:
                nc.vector.tensor_scalar_mul(s_sb[:], s_sb[:], scale)
        elif scale == 1.0:
            nc.vector.tensor_copy(s_sb[:], s_src)
        else:
            nc.vector.tensor_scalar_mul(s_sb[:], s_src, scale)
        p2 = small.tile([JM, nb], BF16, tag="p2")
        nc.vector.tensor_tensor(p2[:], s_sb[:], s_sb[:], mybir.AluOpType.mult)
        sq_ps_t = ps.tile([JM, b_c], F32, tag="s1a")
        sq_ps = sq_ps_t[:J, :nb]
        nc.tensor.matmul(sq_ps, mask_rep[:], p2[:], start=True, stop=True)
        sqs = small.tile([J, nb], F32, tag="sqs")
        nc.vector.tensor_copy(sqs[:], sq_ps)
        rt = small.tile([J, nb], F32, tag="rt")
        nc.scalar.activation(rt[:], sq_ps, mybir.ActivationFunctionType.Sqrt)
        den = small.tile([J, nb], F32, tag="den")
        nc.vector.tensor_scalar_add(den[:], sqs[:], 1.0)
        nc.vector.tensor_tensor(den[:], den[:], rt[:], mybir.AluOpType.mult)
        rden = small.tile([J, nb], F32, tag="rden")
        nc.vector.reciprocal(rden[:], den[:])
        scl = small.tile([J, nb], BF16, tag="scl")
        nc.vector.tensor_tensor(scl[:], sqs[:], rden[:], mybir.AluOpType.mult)
        srep_ps_t = ps.tile([JM, b_c], F32, tag="s1b")
        srep_ps = srep_ps_t[:, :nb]
        nc.tensor.matmul(srep_ps, maskT[:], scl[:], start=True, stop=True)
        srep = small.tile([JM, nb], BF16, tag="srep_sb")
        nc.vector.tensor_copy(srep[:], srep_ps)
        v_jm = small.tile([JM, nb], F32, tag="v_jm")
        nc.vector.tensor_tensor(v_jm[:], s_sb[:], srep[:], mybir.AluOpType.mult)
        return v_jm

    def vblk_fill(vblk_slice, v_jm, nb):
        """vblk_slice [JM, nb, J] <- v_jm [JM, nb] * mask (diag over j)."""
        v16 = small.tile([JM, nb], BF16, tag="v16")
        nc.vector.tensor_copy(v16[:], v_jm[:])
        v_b = bass.AP(
            v16.tensor, v16[:].offset, [v16[:].ap[0], v16[:].ap[1], [0, J]]
        )
        mask_b = bass.AP(
            mask_rep.tensor,
            mask_rep[:].offset,
            [mask_rep[:].ap[0], [0, nb], mask_rep[:].ap[1]],
        )
        nc.gpsimd.tensor_tensor(vblk_slice, v_b, mask_b, mybir.AluOpType.mult)

    for rep in range(reps):
        # ---- iteration 1 (all batches): s1[jm, b] = (1/8) sum_(i,n) W x --
        s1a = ps.tile([JM, b_c], F32, tag="s1a")
        for q in range(H):
            nc.tensor.matmul(
                s1a[:], wcr[:, q, :], xt[:, q, :],
                start=(q == 0), stop=(q == H - 1),
            )
        v1_jm = squash_jm(s1a[:], b_c, 1.0 / J)
        vblk = vbp.tile([JM, b_c, 2, J], BF16, tag="vblk_all")
        vblk_fill(vblk[:, :, 0, :], v1_jm, b_c)

        for hf in range(n_half):
            b0 = hf * half
            # u_hat i-layout resident copy for this half
            u_res = res.tile([128, T, half, JM], BF16, tag="u_res")

            def xb_fetch(t, sp_only=False):
                xb = xs.tile([128, CH_T, half, IP], BF16, tag="xb")
                assert xb_mode == "dram"
                if sp_only:
                    nc.sync.dma_start(xb[:], xblk_d[hf, t])
                else:
                    nc.sync.dma_start(
                        xb[:, : CH_T // 2], xblk_d[hf, t, :, : CH_T // 2]
                    )
                    nc.scalar.dma_start(
                        xb[:, CH_T // 2 :], xblk_d[hf, t, :, CH_T // 2 :]
                    )
                return xb

            def creation(t, xb, pool=None, tag="u_t", dve_copies=(1, 5)):
                u_t = (pool or scr).tile([JM, half, 128], BF16, tag=tag)
                u_tv = u_t[:].rearrange("p b (hh i) -> p hh b i", i=IP)
                cps = None
                for hh in range(CH_T):
                    if hh % 2 == 0:
                        cps = ps2.tile([JM, 2, half, IP], F32, tag="cps")
                    nc.tensor.matmul(
                        cps[:, hh % 2, :, :],
                        wcr[:, t * CH_T + hh, :],
                        xb[:, hh, :, :],
                        start=True,
                        stop=True,
                    )
                    if hh % 2 == 1:
                        pair = u_tv[:, hh - 1 : hh + 1]
                        if hh in dve_copies:
                            nc.vector.tensor_copy(pair, cps[:])
                        else:
                            nc.scalar.activation(
                                pair, cps[:],
                                mybir.ActivationFunctionType.Copy,
                            )
                return u_t

            def fwd_xbar(t, u_t):
                nc.sync.dma_start_transpose(u_res[:, t, :, :], u_t[:, :, :])

            def a_exp(t, u_t, nslot):
                # logits: slot 0 = v1 (it=2); slot 1 = v1+v2 (it=3 --
                # b3 = a1 + a2 = (v1+v2).u_hat by linearity)
                aps = ps3.tile([128, half, J], F32, tag="aps")
                slot = nslot - 1
                for b in range(half):
                    nc.tensor.matmul(
                        aps[:, b, :],
                        u_t[:, b, :],
                        vblk[:, b0 + b, slot, :],
                        start=True,
                        stop=True,
                    )
                e = sm.tile([128, half, J], BF16, tag="e")
                nc.scalar.activation(
                    e[:], aps[:], mybir.ActivationFunctionType.Exp
                )
                return e

            def softmax_norm(e):
                z = sm.tile([128, half], F32, tag="z")
                nc.vector.tensor_reduce(
                    z[:], e[:], mybir.AxisListType.X, mybir.AluOpType.add
                )
                rz = sm.tile([128, half], F32, tag="rz")
                nc.vector.reciprocal(rz[:], z[:])
                c_t = sm.tile([128, half, J], BF16, tag="c_t")
                rzb = bass.AP(
                    rz.tensor, rz[:].offset,
                    [rz[:].ap[0], rz[:].ap[1], [0, J]],
                )
                nc.gpsimd.tensor_tensor(
                    c_t[:], e[:], rzb, mybir.AluOpType.mult
                )
                return c_t

            def s_accum(t, c_t, s_ps):
                for b in range(half):
                    nc.tensor.matmul(
                        s_ps[:, b, :],
                        u_res[:, t, b, :],
                        c_t[:, b, :],
                        start=False,
                        stop=False,
                        skip_group_check=True,
                    )

            def s_extract(s_ps, it):
                # s_ps [JM, half, J]: keep only the j-diagonal block of jm,
                # then sum over j (mask has a single 1 per row).
                msb = smx.tile([JM, half, J], F32, tag="msb")
                mask_b = bass.AP(
                    mask_rep.tensor, mask_rep[:].offset,
                    [mask_rep[:].ap[0], [0, half], mask_rep[:].ap[1]],
                )
                nc.vector.tensor_tensor(
                    msb[:], s_ps[:], mask_b, mybir.AluOpType.mult
                )
                s2_sb = smx.tile([JM, half], F32, tag="s2T")
                nc.vector.tensor_reduce(
                    s2_sb[:], msb[:], mybir.AxisListType.X, mybir.AluOpType.add
                )
                v_jm = squash_jm(s2_sb[:], half, 1.0)
                if it == 2:
                    # store v1+v2 in slot 1 (logit linearity for it=3)
                    vs = small.tile([JM, half], F32, tag="vs")
                    nc.vector.tensor_tensor(
                        vs[:], v_jm[:], v1_jm[:, b0 : b0 + half],
                        mybir.AluOpType.add,
                    )
                    vblk_fill(vblk[:, b0 : b0 + half, 1, :], vs, half)
                else:
                    nc.sync.dma_start(
                        v_out[:]
                        .rearrange("b j m -> (j m) b")[:, b0 : b0 + half],
                        v_jm[:],
                    )

            # ---- it=2 pipelined loop: cr(t) | a(t-1) | s(t-2) ----
            st = {0: {"xb": xb_fetch(0)}, 1: {"xb": xb_fetch(1)}}
            s_ps2 = psS.tile([JM, half, J], F32, tag="s_ps")
            if stage >= 3:
                nc.vector.memset(s_ps2[:], 0.0)
            for t in range(T + 2):
                if t + 2 < T:
                    st[t + 2] = {"xb": xb_fetch(t + 2)}
                if stage >= 2 and 0 <= t - 1 < T:
                    st[t - 1]["e"] = a_exp(t - 1, st[t - 1]["u_t"], 1)
                if t < T:
                    st[t]["u_t"] = creation(t, st[t]["xb"])
                    fwd_xbar(t, st[t]["u_t"])
                if stage >= 3 and 0 <= t - 2 < T:
                    s_accum(t - 2, softmax_norm(st[t - 2]["e"]), s_ps2)
                    del st[t - 2]
            if stage < 2:
                continue
            if stage < 3:
                nc.sync.dma_start(
                    v_out[:].rearrange("b j m -> (j m) b")[:, b0 : b0 + half],
                    v1_jm[:, b0 : b0 + half],
                )
                continue
            s_extract(s_ps2, 2)

            # ---- it=3: rev XBAR (t) | a(t-1) | s(t-2) ----
            def rev_xbar(t):
                u_t3 = scr3.tile([JM, half, 128], BF16, tag="u_t3")
                eng = nc.scalar if t % 2 == 0 else nc.sync
                eng.dma_start_transpose(u_t3[:, :, :], u_res[:, t, :, :])
                return u_t3

            st3 = {}
            if it3_mode != "xbar":
                st3 = {
                    0: {"xb": xb_fetch(0, sp_only=True)},
                    1: {"xb": xb_fetch(1, sp_only=True)},
                }
            s_ps3 = psS.tile([JM, half, J], F32, tag="s_ps")
            nc.vector.memset(s_ps3[:], 0.0)
            for t in range(T + a3_lag + 1):
                if it3_mode != "xbar" and t + 2 < T:
                    st3[t + 2] = {"xb": xb_fetch(t + 2, sp_only=True)}
                if 0 <= t - a3_lag < T:
                    st3[t - a3_lag]["e"] = a_exp(
                        t - a3_lag, st3[t - a3_lag]["u_t"], 2
                    )
                if t < T:
                    if it3_mode == "xbar":
                        st3[t] = {"u_t": rev_xbar(t)}
                    else:
                        st3[t]["u_t"] = creation(
                            t, st3[t]["xb"], pool=scr3, tag="u_t3c"
                        )
                if 0 <= t - a3_lag - 1 < T:
                    s_accum(
                        t - a3_lag - 1, softmax_norm(st3[t - a3_lag - 1]["e"]),
                        s_ps3,
                    )
                    del st3[t - a3_lag - 1]
            s_extract(s_ps3, 3)


_NC_CACHE = {}


def _build_nc(b_c=B_C, half=16, reps=1, stage=3, split=True,
              it3_mode="recreate", xb_mode="dram", scr_bufs=2, a3_lag=2):
    key = (b_c, half, reps, stage, split, it3_mode, xb_mode, scr_bufs, a3_lag)
    if key not in _NC_CACHE:
        _patch_tile()
        nc = bass.Bass("TRN2", target_bir_lowering=False, debug=False)
        wcr_d = nc.dram_tensor("wcr", [128, H, JM], BF16, kind="ExternalInput").ap()
        xt_d = nc.dram_tensor("xt", [128, H, b_c], BF16, kind="ExternalInput").ap()
        mask_d = nc.dram_tensor("mask", [JM, J], BF16, kind="ExternalInput").ap()
        ipm_d = nc.dram_tensor("ipm", [KR, IP], BF16, kind="ExternalInput").ap()
        ins = [wcr_d, xt_d, mask_d, ipm_d]
        if xb_mode == "dram":
            ins.append(
                nc.dram_tensor(
                    "xblk", [b_c // half, T, 128, CH_T, half, IP], BF16,
                    kind="ExternalInput",
                ).ap()
            )
        v_d = nc.dram_tensor("v", [b_c, J, M], F32, kind="ExternalOutput").ap()
        with tile.TileContext(nc) as tc:
            build_kernel(
                tc,
                [v_d],
                ins,
                b_c=b_c,
                half=half,
                reps=reps,
                stage=stage,
                it3_mode=it3_mode,
                xb_mode=xb_mode,
                scr_bufs=scr_bufs,
                a3_lag=a3_lag,
            )
        if split:
            _split_waits(nc)
        _NC_CACHE[key] = nc
    return _NC_CACHE[key]


def host_prep(x, W):
    """Returns (wcr, xt, mask, ipm) host-prepped arrays covering all B.
    Row order of the KR K-rows is (ip, n): i = h*IP + ip."""
    bf = ml_dtypes.bfloat16
    nb = x.shape[0]
    # wcr[(ip*N + n), h, jm] = W[h*IP + ip, j, n, m]
    Wr = np.ascontiguousarray(W.transpose(0, 2, 1, 3)).reshape(I, N, JM)
    Wr = Wr.reshape(H, IP, N, JM)
    wcr = np.ascontiguousarray(Wr.transpose(1, 2, 0, 3)).reshape(KR, H, JM)
    # x rows in the same (ip, n) order per h
    xr = x.reshape(nb, H, IP, N)
    xrows = np.ascontiguousarray(xr.transpose(2, 3, 1, 0)).reshape(KR, H, nb)
    mask = np.zeros((JM, J), np.float32)
    for j in range(J):
        mask[j * M : (j + 1) * M, j] = 1.0
    ipm = np.zeros((KR, IP), np.float32)
    for r in range(KR):
        ipm[r, r // N] = 1.0
    return wcr.astype(bf), xrows.astype(bf), mask.astype(bf), ipm.astype(bf)


def host_blk(x_core, half=16):
    """x [nb, I, N] -> block-diagonal [n_half, T, 128, CH_T, half, IP]."""
    bf = ml_dtypes.bfloat16
    nb = x_core.shape[0]
    xr = np.asarray(x_core, np.float32).reshape(nb, H, IP, N)
    xrows = np.ascontiguousarray(xr.transpose(2, 3, 1, 0)).reshape(KR, H, nb)
    xblk = np.zeros((KR, H, nb, IP), np.float32)
    for r in range(KR):
        xblk[r, :, :, r // N] = xrows[r]
    n_half = nb // half
    xb = xblk.reshape(KR, T, CH_T, nb, IP)
    xb = xb.transpose(3, 1, 0, 2, 4)
    xb = xb.reshape(n_half, half, T, KR, CH_T, IP).transpose(0, 2, 3, 4, 1, 5)
    return np.ascontiguousarray(xb).astype(bf)


def kernel(x, W):
    x = np.asarray(x, np.float32)
    W = np.asarray(W, np.float32)
    wcr, xt_all, mask, ipm = host_prep(x, W)
    nc = _build_nc()
    in_maps = []
    for c in range(N_CORES):
        bs = slice(c * B_C, (c + 1) * B_C)
        in_maps.append(
            {
                "wcr": wcr,
                "xt": np.ascontiguousarray(xt_all[:, :, bs]),
                "mask": mask,
                "ipm": ipm,
                "xblk": host_blk(x[bs]),
            }
        )
    res = run_bass_kernel_spmd(nc, in_maps, list(range(N_CORES)))
    out = np.concatenate([res.results[c]["v"] for c in range(N_CORES)], axis=0)
    return out.astype(np.float32)


# revision 31
# speedup vs baseline: 2.8610x; 2.8610x over previous
"""DigitCaps dynamic-routing kernel for Trainium2 (8 NeuronCores, Bass/Tile).

Problem: B=256, IN_CAPS=3200, IN_DIM=8, OUT_CAPS=8, OUT_DIM=16, 3 routing
iterations.  Data-parallel over batch: 32 batches per core.

Per core (v2 design):
  - u_hat is created ONCE per 16-batch half via full-K=128 matmuls over a
    host-prepared block-diagonal x operand ([jm=128-partition, (b, i)]
    layout, bf16), then XBAR-transposed into the resident i-layout copy
    u_res [i, t, b, jm].
  - it=2 a-pass uses the creation-staging jm-layout tiles directly;
    it=3 rebuilds jm-layout tiles from u_res via reverse XBAR transposes
    (this replaces the baseline's second full creation pass: saves the
    second xblk DMA read, 400 matmuls and 200 PSUM->SBUF copies).
  - squash runs entirely in [jm, b] layout: sq comes from a mask matmul
    (contract m within each j block), the j-indexed scale is re-broadcast
    across jm partitions with the transposed mask matmul.  No identity
    transposes, no per-squash XBAR.
  - PE emission is software-pipelined: creation(t) | a-pass(t-1) |
    s-pass(t-2) so PE never waits on the copy/softmax chains.
  - PSUM->SBUF creation copies rotate over Pool/Pool/DVE/Act; XBAR and
    xblk DMAs alternate between the SP and Act hardware DGE queues.
"""

import sys

if "/opt/trn_rl_repo" not in sys.path:
    sys.path.insert(0, "/opt/trn_rl_repo")

import ml_dtypes
import numpy as np

import bass_rust
import concourse.bass as bass
import concourse.mybir as mybir
import concourse.tile as tile
from concourse._compat import with_exitstack
from concourse.bass_utils import run_bass_kernel_spmd
from concourse.vector_clock import ScopedClock

# ---------------------------------------------------------------------------
# Walrus on this toolchain rejects multi-wait CTRL instructions;
# TileContext's tail drain aggregates one wait per outstanding semaphore.
# Split the waits across consecutive SP drains.
_TILE_PATCHED = False


def _drain_and_barrier_split(self, tick_clock, wait_clock):
    drain_inst = self.nc.sync.drain()
    wait_clock.add_sem_waits(
        drain_inst.ins, ScopedClock({None: tick_clock.global_clock})
    )
    mi = drain_inst.ins
    waits = list(mi.sync_info.on_wait) if mi.sync_info else []
    if len(waits) > 1:
        si = mi.sync_info
        si.on_wait = waits[:1]
        mi.sync_info = si
        for i in range(1, len(waits)):
            extra = self.nc.sync.drain().ins
            extra.sync_info = bass_rust.SyncInfo(
                on_wait=waits[i : i + 1], on_update=[]
            )
    self.nc.all_engine_barrier()
    assert self.sems is not None
    popped = self.nc._tile_sem_poison_stack.pop()
    assert popped is self._sem_poison
    self.nc.clear_and_free_semaphores(list(self.sems.allocated().values()))
    self.nc.all_engine_barrier()


def _patch_tile():
    global _TILE_PATCHED
    if not _TILE_PATCHED:
        tile.TileContext._drain_and_barrier = _drain_and_barrier_split
        _TILE_PATCHED = True


_SW_COUNT = [0]


def _split_waits(nc):
    """This walrus build allows one sync wait per instruction: hoist extra
    waits onto same-engine NoOp carriers placed just before."""
    for f in nc.m.functions:
        for blk in f.blocks:
            insts = blk.instructions
            if not any(
                inst.sync_info and len(inst.sync_info.on_wait) > 1
                for inst in insts
            ):
                continue
            new = []
            for inst in insts:
                si = inst.sync_info
                waits = list(si.on_wait) if si else []
                if len(waits) > 1:
                    for w in waits[:-1]:
                        _SW_COUNT[0] += 1
                        car = mybir.InstNoOp(
                            name=f"I-sw{_SW_COUNT[0]}", engine=inst.engine
                        )
                        car.sync_info = bass_rust.SyncInfo(
                            on_wait=[w], on_update=[]
                        )
                        new.append(car)
                    si.on_wait = waits[-1:]
                    inst.sync_info = si
                new.append(inst)
            insts[:] = new


# ---------------------------------------------------------------------------
B, I, N, J, M = 256, 3200, 8, 8, 16
JM = J * M  # 128
N_CORES = 8
B_C = B // N_CORES  # 32
T = I // 128  # 25 i-tiles

IP = 16  # i's packed per K-chunk (K = IP*N = 128, uniform row group)
KR = IP * N  # 128 K-rows per chunk
H = I // IP  # 200
CH_T = 128 // IP  # 8 creation chunks per 128-i tile

F32 = mybir.dt.float32
BF16 = mybir.dt.bfloat16


@with_exitstack
def build_kernel(ctx, tc, outs, ins, b_c=B_C, half=16, reps=1, stage=3,
                 it3_mode="xbar", xb_mode="chip", scr_bufs=2, a3_lag=2):
    """stage: 1=creation only, 2=+a+softmax, 3=full (timing ablation)."""
    nc = tc.nc
    (v_out,) = outs
    if xb_mode == "dram":
        (wcr_d, xt_d, mask_d, ipm_d, xblk_d) = ins
    else:
        (wcr_d, xt_d, mask_d, ipm_d) = ins
    n_half = b_c // half

    const = ctx.enter_context(tc.tile_pool(name="const", bufs=1))
    res = ctx.enter_context(tc.tile_pool(name="res", bufs=1))
    scr = ctx.enter_context(tc.tile_pool(name="scr", bufs=scr_bufs))
    scr3 = ctx.enter_context(tc.tile_pool(name="scr3", bufs=3))
    xs = ctx.enter_context(tc.tile_pool(name="xs", bufs=3))
    sm = ctx.enter_context(tc.tile_pool(name="sm", bufs=3))
    small = ctx.enter_context(tc.tile_pool(name="small", bufs=2))
    vbp = ctx.enter_context(tc.tile_pool(name="vbp", bufs=1))
    smx = ctx.enter_context(tc.tile_pool(name="smx", bufs=1))
    ps = ctx.enter_context(tc.tile_pool(name="ps", bufs=1, space="PSUM"))
    psS = ctx.enter_context(tc.tile_pool(name="psS", bufs=1, space="PSUM"))
    ps2 = ctx.enter_context(tc.tile_pool(name="ps2", bufs=2, space="PSUM"))
    ps3 = ctx.enter_context(tc.tile_pool(name="ps3", bufs=2, space="PSUM"))

    # Resident constants (K = 128 rows; all matmul bases stay 0 -- any mix
    # of stationary base partitions crashes this hardware).
    wcr = const.tile([128, H, JM], BF16)
    for k in range(4):
        eng = nc.sync if k % 2 == 0 else nc.scalar
        nc_q = H // 4
        eng.dma_start(
            wcr[:, k * nc_q : (k + 1) * nc_q, :],
            wcr_d[:, k * nc_q : (k + 1) * nc_q, :],
        )
    xt = const.tile([128, H, b_c], BF16)
    for k in range(2):
        eng = nc.sync if k % 2 == 0 else nc.scalar
        nc_q = H // 2
        eng.dma_start(
            xt[:, k * nc_q : (k + 1) * nc_q, :],
            xt_d[:, k * nc_q : (k + 1) * nc_q, :],
        )
    mask_rep = const.tile([JM, J], BF16)
    nc.sync.dma_start(mask_rep[:], mask_d[:])
    maskT = const.tile([J, JM], BF16)
    nc.sync.dma_start(maskT[:], mask_d[:].rearrange("a b -> b a"))
    if xb_mode == "chip2":
        ipm = const.tile([KR, half, IP], BF16)
        ipm_src = bass.AP(
            ipm_d.tensor, ipm_d.offset,
            [ipm_d.ap[0], [0, half], ipm_d.ap[1]],
        )
        nc.sync.dma_start(ipm[:], ipm_src)
    else:
        ipm = const.tile([KR, IP], BF16)
        nc.sync.dma_start(ipm[:], ipm_d[:])
    lg_res = const.tile([128, T, half, J], BF16)

    def squash_jm(s_src, nb, scale, s_src2=None):
        """s_src [JM, nb] f32 (PSUM/SBUF) (+ optional second PSUM operand)
        -> v_jm [JM, nb] f32 SBUF (squashed)."""
        s_sb = small.tile([JM, nb], F32, tag="s_sb")
        if s_src2 is not None:
            nc.vector.tensor_copy(s_sb[:], s_src)
            nc.vector.tensor_tensor(
                s_sb[:], s_sb[:], s_src2, mybir.AluOpType.add
            )
            if scale != 1.0:
                nc.vector.tensor_scalar_mul(s_sb[:], s_sb[:], scale)
        elif scale == 1.0:
            nc.vector.tensor_copy(s_sb[:], s_src)
        else:
            nc.vector.tensor_scalar_mul(s_sb[:], s_src, scale)
        p2 = small.tile([JM, nb], BF16, tag="p2")
        nc.vector.tensor_tensor(p2[:], s_sb[:], s_sb[:], mybir.AluOpType.mult)
        sq_ps_t = ps.tile([JM, b_c], F32, tag="s1a")
        sq_ps = sq_ps_t[:J, :nb]
        nc.tensor.matmul(sq_ps, mask_rep[:], p2[:], start=True, stop=True)
        sqs = small.tile([J, nb], F32, tag="sqs")
        nc.vector.tensor_copy(sqs[:], sq_ps)
        rt = small.tile([J, nb], F32, tag="rt")
        nc.scalar.activation(rt[:], sq_ps, mybir.ActivationFunctionType.Sqrt)
        den = small.tile([J, nb], F32, tag="den")
        nc.vector.tensor_scalar_add(den[:], sqs[:], 1.0)
        nc.vector.tensor_tensor(den[:], den[:], rt[:], mybir.AluOpType.mult)
        rden = small.tile([J, nb], F32, tag="rden")
        nc.vector.reciprocal(rden[:], den[:])
        scl = small.tile([J, nb], BF16, tag="scl")
        nc.vector.tensor_tensor(scl[:], sqs[:], rden[:], mybir.AluOpType.mult)
        srep_ps_t = ps.tile([JM, b_c], F32, tag="s1b")
        srep_ps = srep_ps_t[:, :nb]
        nc.tensor.matmul(srep_ps, maskT[:], scl[:], start=True, stop=True)
        srep = small.tile([JM, nb], BF16, tag="srep_sb")
        nc.vector.tensor_copy(srep[:], srep_ps)
        v_jm = small.tile([JM, nb], F32, tag="v_jm")
        nc.vector.tensor_tensor(v_jm[:], s_sb[:], srep[:], mybir.AluOpType.mult)
        return v_jm

    def vblk_fill(vblk_slice, v_jm, nb):
        """vblk_slice [JM, nb, J] <- v_jm [JM, nb] * mask (diag over j)."""
        v16 = small.tile([JM, nb], BF16, tag="v16")
        nc.vector.tensor_copy(v16[:], v_jm[:])
        v_b = bass.AP(
            v16.tensor, v16[:].offset, [v16[:].ap[0], v16[:].ap[1], [0, J]]
        )
        mask_b = bass.AP(
            mask_rep.tensor,
            mask_rep[:].offset,
            [mask_rep[:].ap[0], [0, nb], mask_rep[:].ap[1]],
        )
        nc.gpsimd.tensor_tensor(vblk_slice, v_b, mask_b, mybir.AluOpType.mult)

    for rep in range(reps):
        # ---- iteration 1 (all batches): s1[jm, b] = (1/8) sum_(i,n) W x --
        s1a = ps.tile([JM, b_c], F32, tag="s1a")
        for q in range(H):
            nc.tensor.matmul(
                s1a[:], wcr[:, q, :], xt[:, q, :],
                start=(q == 0), stop=(q == H - 1),
            )
        v1_jm = squash_jm(s1a[:], b_c, 1.0 / J)
        vblk = vbp.tile([JM, b_c, 2, J], BF16, tag="vblk_all")
        vblk_fill(vblk[:, :, 0, :], v1_jm, b_c)

        for hf in range(n_half):
            b0 = hf * half
            # u_hat i-layout resident copy for this half
            u_res = res.tile([128, T, half, JM], BF16, tag="u_res")

            def xb_fetch(t, sp_only=False):
                xb = xs.tile([128, CH_T, half, IP], BF16, tag="xb")
                assert xb_mode == "dram"
                if sp_only:
                    nc.sync.dma_start(xb[:], xblk_d[hf, t])
                else:
                    nc.sync.dma_start(
                        xb[:, : CH_T // 2], xblk_d[hf, t, :, : CH_T // 2]
                    )
                    nc.scalar.dma_start(
                        xb[:, CH_T // 2 :], xblk_d[hf, t, :, CH_T // 2 :]
                    )
                return xb

            def creation(t, xb, pool=None, tag="u_t", dve_copies=(1, 5)):
                u_t = (pool or scr).tile([JM, half, 128], BF16, tag=tag)
                u_tv = u_t[:].rearrange("p b (hh i) -> p hh b i", i=IP)
                cps = None
                for hh in range(CH_T):
                    if hh % 2 == 0:
                        cps = ps2.tile([JM, 2, half, IP], F32, tag="cps")
                    nc.tensor.matmul(
                        cps[:, hh % 2, :, :],
                        wcr[:, t * CH_T + hh, :],
                        xb[:, hh, :, :],
                        start=True,
                        stop=True,
                    )
                    if hh % 2 == 1:
                        pair = u_tv[:, hh - 1 : hh + 1]
                        if hh in dve_copies:
                            nc.vector.tensor_copy(pair, cps[:])
                        else:
                            nc.scalar.activation(
                                pair, cps[:],
                                mybir.ActivationFunctionType.Copy,
                            )
                return u_t

            def fwd_xbar(t, u_t):
                nc.sync.dma_start_transpose(u_res[:, t, :, :], u_t[:, :, :])

            def a_exp(t, u_t, nslot):
                # logits: slot 0 = v1 (it=2); slot 1 = v1+v2 (it=3 --
                # b3 = a1 + a2 = (v1+v2).u_hat by linearity)
                aps = ps3.tile([128, half, J], F32, tag="aps")
                slot = nslot - 1
                for b in range(half):
                    nc.tensor.matmul(
                        aps[:, b, :],
                        u_t[:, b, :],
                        vblk[:, b0 + b, slot, :],
                        start=True,
                        stop=True,
                    )
                e = sm.tile([128, half, J], BF16, tag="e")
                nc.scalar.activation(
                    e[:], aps[:], mybir.ActivationFunctionType.Exp
                )
                return e

            def softmax_norm(e):
                z = sm.tile([128, half], F32, tag="z")
                nc.vector.tensor_reduce(
                    z[:], e[:], mybir.AxisListType.X, mybir.AluOpType.add
                )
                rz = sm.tile([128, half], F32, tag="rz")
                nc.vector.reciprocal(rz[:], z[:])
                c_t = sm.tile([128, half, J], BF16, tag="c_t")
                rzb = bass.AP(
                    rz.tensor, rz[:].offset,
                    [rz[:].ap[0], rz[:].ap[1], [0, J]],
                )
                nc.gpsimd.tensor_tensor(
                    c_t[:], e[:], rzb, mybir.AluOpType.mult
                )
                return c_t

            def s_accum(t, c_t, s_ps):
                for b in range(half):
                    nc.tensor.matmul(
                        s_ps[:, b, :],
                        u_res[:, t, b, :],
                        c_t[:, b, :],
                        start=False,
                        stop=False,
                        skip_group_check=True,
                    )

            def s_extract(s_ps, it):
                # s_ps [JM, half, J]: keep only the j-diagonal block of jm,
                # then sum over j (mask has a single 1 per row).
                msb = smx.tile([JM, half, J], F32, tag="msb")
                mask_b = bass.AP(
                    mask_rep.tensor, mask_rep[:].offset,
                    [mask_rep[:].ap[0], [0, half], mask_rep[:].ap[1]],
                )
                nc.vector.tensor_tensor(
                    msb[:], s_ps[:], mask_b, mybir.AluOpType.mult
                )
                s2_sb = smx.tile([JM, half], F32, tag="s2T")
                nc.vector.tensor_reduce(
                    s2_sb[:], msb[:], mybir.AxisListType.X, mybir.AluOpType.add
                )
                v_jm = squash_jm(s2_sb[:], half, 1.0)
                if it == 2:
                    # store v1+v2 in slot 1 (logit linearity for it=3)
                    vs = small.tile([JM, half], F32, tag="vs")
                    nc.vector.tensor_tensor(
                        vs[:], v_jm[:], v1_jm[:, b0 : b0 + half],
                        mybir.AluOpType.add,
                    )
                    vblk_fill(vblk[:, b0 : b0 + half, 1, :], vs, half)
                else:
                    nc.sync.dma_start(
                        v_out[:]
                        .rearrange("b j m -> (j m) b")[:, b0 : b0 + half],
                        v_jm[:],
                    )

            # ---- it=2 pipelined loop: cr(t) | a(t-1) | s(t-2) ----
            st = {0: {"xb": xb_fetch(0)}, 1: {"xb": xb_fetch(1)}}
            s_ps2 = psS.tile([JM, half, J], F32, tag="s_ps")
            if stage >= 3:
                nc.vector.memset(s_ps2[:], 0.0)
            for t in range(T + 2):
                if t + 2 < T:
                    st[t + 2] = {"xb": xb_fetch(t + 2)}
                if stage >= 2 and 0 <= t - 1 < T:
                    st[t - 1]["e"] = a_exp(t - 1, st[t - 1]["u_t"], 1)
                if t < T:
                    st[t]["u_t"] = creation(t, st[t]["xb"])
                    fwd_xbar(t, st[t]["u_t"])
                if stage >= 3 and 0 <= t - 2 < T:
                    s_accum(t - 2, softmax_norm(st[t - 2]["e"]), s_ps2)
                    del st[t - 2]
            if stage < 2:
                continue
            if stage < 3:
                nc.sync.dma_start(
                    v_out[:].rearrange("b j m -> (j m) b")[:, b0 : b0 + half],
                    v1_jm[:, b0 : b0 + half],
                )
                continue
            s_extract(s_ps2, 2)

            # ---- it=3: rev XBAR (t) | a(t-1) | s(t-2) ----
            def rev_xbar(t):
                u_t3 = scr3.tile([JM, half, 128], BF16, tag="u_t3")
                eng = nc.scalar if t % 2 == 0 else nc.sync
                eng.dma_start_transpose(u_t3[:, :, :], u_res[:, t, :, :])
                return u_t3

            st3 = {}
            if it3_mode != "xbar":
                st3 = {
                    0: {"xb": xb_fetch(0, sp_only=True)},
                    1: {"xb": xb_fetch(1, sp_only=True)},
                }
            s_ps3 = psS.tile([JM, half, J], F32, tag="s_ps")
            nc.vector.memset(s_ps3[:], 0.0)
            for t in range(T + a3_lag + 1):
                if it3_mode != "xbar" and t + 2 < T:
                    st3[t + 2] = {"xb": xb_fetch(t + 2, sp_only=True)}
                if 0 <= t - a3_lag < T:
                    st3[t - a3_lag]["e"] = a_exp(
                        t - a3_lag, st3[t - a3_lag]["u_t"], 2
                    )
                if t < T:
                    if it3_mode == "xbar":
                        st3[t] = {"u_t": rev_xbar(t)}
                    else:
                        st3[t]["u_t"] = creation(
                            t, st3[t]["xb"], pool=scr3, tag="u_t3c"
                        )
                if 0 <= t - a3_lag - 1 < T:
                    s_accum(
                        t - a3_lag - 1, softmax_norm(st3[t - a3_lag - 1]["e"]),
                        s_ps3,
                    )
                    del st3[t - a3_lag - 1]
            s_extract(s_ps3, 3)


_NC_CACHE = {}


def _build_nc(b_c=B_C, half=16, reps=1, stage=3, split=True,
              it3_mode="recreate", xb_mode="dram", scr_bufs=2, a3_lag=2):
    key = (b_c, half, reps, stage, split, it3_mode, xb_mode, scr_bufs, a3_lag)
    if key not in _NC_CACHE:
        _patch_tile()
        nc = bass.Bass("TRN2", target_bir_lowering=False, debug=False)
        wcr_d = nc.dram_tensor("wcr", [128, H, JM], BF16, kind="ExternalInput").ap()
        xt_d = nc.dram_tensor("xt", [128, H, b_c], BF16, kind="ExternalInput").ap()
        mask_d = nc.dram_tensor("mask", [JM, J], BF16, kind="ExternalInput").ap()
        ipm_d = nc.dram_tensor("ipm", [KR, IP], BF16, kind="ExternalInput").ap()
        ins = [wcr_d, xt_d, mask_d, ipm_d]
        if xb_mode == "dram":
            ins.append(
                nc.dram_tensor(
                    "xblk", [b_c // half, T, 128, CH_T, half, IP], BF16,
                    kind="ExternalInput",
                ).ap()
            )
        v_d = nc.dram_tensor("v", [b_c, J, M], F32, kind="ExternalOutput").ap()
        with tile.TileContext(nc) as tc:
            build_kernel(
                tc,
                [v_d],
                ins,
                b_c=b_c,
                half=half,
                reps=reps,
                stage=stage,
                it3_mode=it3_mode,
                xb_mode=xb_mode,
                scr_bufs=scr_bufs,
                a3_lag=a3_lag,
            )
        if split:
            _split_waits(nc)
        _NC_CACHE[key] = nc
    return _NC_CACHE[key]


def host_prep(x, W):
    """Returns (wcr, xt, mask, ipm) host-prepped arrays covering all B.
    Row order of the KR K-rows is (ip, n): i = h*IP + ip."""
    bf = ml_dtypes.bfloat16
    nb = x.shape[0]
    # wcr[(ip*N + n), h, jm] = W[h*IP + ip, j, n, m]
    Wr = np.ascontiguousarray(W.transpose(0, 2, 1, 3)).reshape(I, N, JM)
    Wr = Wr.reshape(H, IP, N, JM)
    wcr = np.ascontiguousarray(Wr.transpose(1, 2, 0, 3)).reshape(KR, H, JM)
    # x rows in the same (ip, n) order per h
    xr = x.reshape(nb, H, IP, N)
    xrows = np.ascontiguousarray(xr.transpose(2, 3, 1, 0)).reshape(KR, H, nb)
    mask = np.zeros((JM, J), np.float32)
    for j in range(J):
        mask[j * M : (j + 1) * M, j] = 1.0
    ipm = np.zeros((KR, IP), np.float32)
    for r in range(KR):
        ipm[r, r // N] = 1.0
    return wcr.astype(bf), xrows.astype(bf), mask.astype(bf), ipm.astype(bf)


def host_blk(x_core, half=16):
    """x [nb, I, N] -> block-diagonal [n_half, T, 128, CH_T, half, IP]."""
    bf = ml_dtypes.bfloat16
    nb = x_core.shape[0]
    xr = np.asarray(x_core, np.float32).reshape(nb, H, IP, N)
    xrows = np.ascontiguousarray(xr.transpose(2, 3, 1, 0)).reshape(KR, H, nb)
    xblk = np.zeros((KR, H, nb, IP), np.float32)
    for r in range(KR):
        xblk[r, :, :, r // N] = xrows[r]
    n_half = nb // half
    xb = xblk.reshape(KR, T, CH_T, nb, IP)
    xb = xb.transpose(3, 1, 0, 2, 4)
    xb = xb.reshape(n_half, half, T, KR, CH_T, IP).transpose(0, 2, 3, 4, 1, 5)
    return np.ascontiguousarray(xb).astype(bf)


def kernel(x, W):
    x = np.asarray(x, np.float32)
    W = np.asarray(W, np.float32)
    wcr, xt_all, mask, ipm = host_prep(x, W)
    nc = _build_nc()
    in_maps = []
    for c in range(N_CORES):
        bs = slice(c * B_C, (c + 1) * B_C)
        in_maps.append(
            {
                "wcr": wcr,
                "xt": np.ascontiguousarray(xt_all[:, :, bs]),
                "mask": mask,
                "ipm": ipm,
                "xblk": host_blk(x[bs]),
            }
        )
    res = run_bass_kernel_spmd(nc, in_maps, list(range(N_CORES)))
    out = np.concatenate([res.results[c]["v"] for c in range(N_CORES)], axis=0)
    return out.astype(np.float32)
